# revision 9
# baseline (speedup 1.0000x reference)
"""BinaryTreeLSTM Trainium2 kernel — data-parallel over the batch (tree) axis.

v2: fp16 datapath + bit-reversed position-major layout.

Layout: the host permutes each tree's 256 leaves into bit-reversed order
and interleaves the 8 trees of a chunk (column = position*8 + tree).  With
bit-reversed storage, the children of the node at position p of level l+1
are at positions (p, p + N/2) of level l — so every level's lh/rh operands
are *contiguous halves* instead of stride-2 slices, and the level outputs
are written contiguously.  This keeps every DVE tensor_tensor op on dense
step-1 fp16 data (2x_1P mode) and every matmul moving operand dense.

dtypes: all matmul operands, gates, h and c are fp16 (e5m10) — ~4x lower
end-to-end error than the bf16 variant (sim: 3.0e-3 vs 1.14e-2) because
the c-accumulation chain keeps 10 mantissa bits.  PSUM and biases fp32.

Per core (64 trees): leaf GEMM (K=300 -> 512 feats = [c|o]), then 8
levelized compose GEMMs (K=512 = [lh|rh] feats -> 1024 = 4 gates x 256).
Chunks of 8 trees run leaf+L1..L3 software-pipelined (leaf ch | L2 ch-1 |
L1 ch | L3 ch-2); L3 parks position-major per 32-tree group; the group-0
tail (L4..L8) interleaves with chunks 6-7 so only group 1's tail runs
after the main loop.

Startup: DVE memset -> dummy sigmoid (preloads the sigmoid+tanh ACT table
set during the first DMA wait) -> dummy matmuls (PE HAM warm-up); the
first leaf sub-chunk's embs DMA is split into 4 pieces so the first real
matmul starts as early as possible.
"""

import sys

if "/opt/trn_rl_repo" not in sys.path:
    sys.path.insert(0, "/opt/trn_rl_repo")

from contextlib import ExitStack

import numpy as np

N_CORES = 8
B, L, IN, M = 512, 256, 300, 256
BC = B // N_CORES            # trees per core
LEAF_ROWS = BC * L           # 16384 leaf columns per core
T_CHUNK = 8                  # trees per chunk
N_CHUNKS = BC // T_CHUNK     # 8
CHUNK_LEAF = T_CHUNK * L     # 2048 leaf columns per chunk

_CACHE = {}
LAST_RESULTS = None


def _build():
    import concourse.bass as bass  # noqa: F401
    import concourse.tile as tile
    from concourse import bacc, mybir

    F32 = mybir.dt.float32
    F16 = mybir.dt.float16
    SIG = mybir.ActivationFunctionType.Sigmoid
    TANH = mybir.ActivationFunctionType.Tanh
    ADD = mybir.AluOpType.add

    nc = bacc.Bacc("TRN2", target_bir_lowering=False, debug=False,
                   num_devices=N_CORES)
    emb_d = nc.dram_tensor("embs_t", [IN, LEAF_ROWS], F16,
                           kind="ExternalInput").ap()
    wl_d = nc.dram_tensor("w_leaf", [IN, 2 * M], F16,
                          kind="ExternalInput").ap()
    wc_d = nc.dram_tensor("w_comp", [2 * M, 4 * M], F16,
                          kind="ExternalInput").ap()
    wl2_d = nc.dram_tensor("w_leaf2", [128, 2 * M], F16,
                           kind="ExternalInput").ap()
    b_d = nc.dram_tensor("biases", [128, 12], F32, kind="ExternalInput").ap()
    outc_d = nc.dram_tensor("out_c", [M, BC], F32, kind="ExternalOutput").ap()
    outh_d = nc.dram_tensor("out_h", [M, BC], F32, kind="ExternalOutput").ap()

    with tile.TileContext(nc) as tc, ExitStack() as ctx:
        wpool = ctx.enter_context(tc.tile_pool(name="w", bufs=1))
        xpool = ctx.enter_context(tc.tile_pool(name="x", bufs=5))
        lvl = ctx.enter_context(tc.tile_pool(name="lvl", bufs=1))
        gp = ctx.enter_context(tc.tile_pool(name="g", bufs=2))
        pp = ctx.enter_context(tc.tile_pool(name="ps", bufs=1, space="PSUM"))

        # --- weights / biases (resident); issued on the ACT HWDGE queue so
        # the SP queue leads with the first leaf's embs tiles ---
        wl = [wpool.tile([128, 2 * M], F16, name=f"wl{k}", tag=f"wl{k}")
              for k in range(2)]
        nc.scalar.dma_start(wl[0][:], wl_d[0:128, :])
        nc.scalar.dma_start(wl[1][:], wl_d[128:256, :])
        wl2 = wpool.tile([128, 2 * M], F16, name="wl2", tag="wl2")
        nc.scalar.dma_start(wl2[:], wl2_d[:, :])
        wc = [wpool.tile([128, 4 * M], F16, name=f"wc{k}", tag=f"wc{k}")
              for k in range(4)]
        bias = wpool.tile([128, 12], F32, name="bias", tag="bias")
        nc.scalar.dma_start(bias[:], b_d[:, :])

        # PSUM: eight [128, 512] tiles = one bank each
        def ps_tile(i, n=512):
            t = pp.tile([128, 512], F32, name=f"ps{i}", tag=f"ps{i}")
            return t[:, :n] if n < 512 else t

        # --- warm-up: memset on DVE; dummy sigmoid preloads the
        # sigmoid_and_others ACT table set (covers tanh too, ~2.7us)
        # during the startup DMA wait; dummy matmuls on banks 4-7 engage
        # the PE HAM clock gate ---
        dummy = wpool.tile([128, 512], F16, name="dummy", tag="dummy")
        nc.vector.memset(dummy[:], 0.0)
        actpre = wpool.tile([128, 1], F32, name="actpre", tag="actpre")
        nc.scalar.activation(actpre[:], dummy[:, 0:1], SIG)
        for i in range(7):
            nc.tensor.matmul(ps_tile(4 + i % 4), dummy[:, 0:128],
                             dummy[:], start=True, stop=True)

        # --- parked L3 state: position-major per tree-group:
        # col = half*(32*T) + pos*T + tree_in_group.  Unbalanced groups
        # (chunks 0-3 / 4-6 / 7) so the last group's L4..L8 tail — the
        # only one that can't hide under the chunk loop — is tiny and
        # interleaves with group 1's calls.
        GRP_CH = ((0, 1, 2, 3), (4, 5, 6), (7,))
        GRP_T = tuple(len(cs) * T_CHUNK for cs in GRP_CH)   # (32, 24, 8)
        GRP_BASE = (0, 32, 56)
        pk_h = [lvl.tile([128, 64 * t], F16, name=f"pkh{g}", tag=f"pkh{g}")
                for g, t in enumerate(GRP_T)]
        pk_c = [lvl.tile([128, 64 * t], F16, name=f"pkc{g}", tag=f"pkc{g}")
                for g, t in enumerate(GRP_T)]

        # compose m-tile index: mt = gate*2 + part_half, gates (i,lf,rf,u).
        # Drain order (i0,i1,u0,u1,...) lets the DVE c-chain start after
        # four gate activations.
        MT_ORDER = (0, 1, 6, 7, 2, 3, 4, 5)
        GATE_OF_MT = (0, 0, 1, 1, 2, 2, 3, 3)

        def compose(rhs, lc, rc, couts, n):
            """One compose call (n <= 512 output nodes).

            rhs: 4 moving APs [128, n] (lh p0, lh p1, rh p0, rh p1);
            lc/rc: [128, 2, n] APs; couts: list of 2 per-half out APs.

            The k-tiles run in two phases (p0 features of both children,
            then p1) so this call only needs the previous level's p0-half
            h to start — the producer's p1 chain hides under phase A."""
            ps = [ps_tile(mt, n) for mt in range(8)]
            for ka, kb in ((0, 2), (1, 3)):
                for mt in MT_ORDER:
                    nc.tensor.matmul(ps[mt],
                                     wc[ka][:, mt * 128:(mt + 1) * 128],
                                     rhs[ka], start=(ka == 0), stop=False)
                    nc.tensor.matmul(ps[mt],
                                     wc[kb][:, mt * 128:(mt + 1) * 128],
                                     rhs[kb], start=False, stop=(kb == 3))
            g = [None] * 4
            for mt in MT_ORDER:
                gate, p = mt >> 1, mt & 1
                if g[gate] is None:
                    g[gate] = gp.tile([128, 2, 512], F16, name=f"g{gate}",
                                      tag=f"g{gate}", bufs=3)
                fn = TANH if gate == 3 else SIG
                nc.scalar.activation(g[gate][:, p, :n], ps[mt], fn,
                                     bias=bias[:, 4 + mt:5 + mt])
            t1 = gp.tile([128, 2, 512], F16, name="t1", tag="t1", bufs=2)
            nc.vector.tensor_mul(t1[:, :, :n], g[0][:, :, :n],
                                 g[3][:, :, :n])
            t2 = gp.tile([128, 2, 512], F16, name="t2", tag="t2", bufs=2)
            nc.vector.tensor_mul(t2[:, :, :n], g[1][:, :, :n], lc)
            t3 = gp.tile([128, 2, 512], F16, name="t3", tag="t3", bufs=2)
            nc.vector.tensor_mul(t3[:, :, :n], g[2][:, :, :n], rc)
            s12 = gp.tile([128, 2, 512], F16, name="s12", tag="s12", bufs=2)
            nc.vector.tensor_add(s12[:, :, :n], t1[:, :, :n], t2[:, :, :n])
            for p in range(2):
                a, b = s12[:, p, :n], t3[:, p, :n]
                if couts[p].ndim == 3:  # park layout [128, pos, tree]
                    td = couts[p].shape[2]
                    a = a.rearrange("q (a b) -> q a b", b=td)
                    b = b.rearrange("q (a b) -> q a b", b=td)
                nc.vector.tensor_add(couts[p], a, b)

        st = {}  # (level, ch) -> (h_tile, c_tile)

        def emit_leaf(ch):
            h_lf = lvl.tile([128, 4096], F16, name="hlf", tag="hlf", bufs=2)
            c_lf = lvl.tile([128, 4096], F16, name="clf", tag="clf", bufs=2)
            for s in range(4):
                col0 = ch * CHUNK_LEAF + s * 512
                xk01 = xpool.tile([128, 2, 512], F16, name="xk01",
                                  tag="xk01")
                nc.sync.dma_start(xk01[:, 0, :], emb_d[0:128, col0:col0 + 512])
                nc.sync.dma_start(xk01[:, 1, :],
                                  emb_d[128:256, col0:col0 + 512])
                xk2 = xpool.tile([128, 512], F16, name="xk2", tag="xk2")
                nc.sync.dma_start(xk2[0:44, :],
                                  emb_d[256:300, col0:col0 + 512])
                nc.sync.dma_start(xk2[64:108, :],
                                  emb_d[256:300, col0:col0 + 512])
                pso = 4 * (s % 2)
                ps = [ps_tile(pso + mt) for mt in range(4)]
                # k-outer so the first matmuls only need the first DMA piece
                for k in range(2):
                    for mt in range(4):
                        nc.tensor.matmul(
                            ps[mt], wl[k][:, mt * 128:(mt + 1) * 128],
                            xk01[:, k, :], start=(k == 0), stop=False)
                # K=44 tail: two m-tiles in disjoint PE row groups
                for mtp in (0, 2):
                    nc.tensor.matmul(
                        ps[mtp], wl2[0:44, mtp * 128:(mtp + 1) * 128],
                        xk2[0:44, :], start=False, stop=True,
                        tile_position=(0, 0))
                    nc.tensor.matmul(
                        ps[mtp + 1],
                        wl2[64:108, (mtp + 1) * 128:(mtp + 2) * 128],
                        xk2[64:108, :], start=False, stop=True,
                        tile_position=(64, 0))
                tcell = gp.tile([128, 1024], F16, name="lftc", tag="lftc",
                                bufs=2)
                to = gp.tile([128, 1024], F16, name="lfto", tag="lfto",
                             bufs=2)
                for p in range(2):
                    nc.scalar.activation(tcell[:, p * 512:(p + 1) * 512],
                                         ps[p], TANH, bias=bias[:, p:p + 1])
                    nc.scalar.activation(to[:, p * 512:(p + 1) * 512],
                                         ps[2 + p], SIG,
                                         bias=bias[:, 2 + p:3 + p])
                    nc.vector.tensor_scalar(
                        c_lf[:, s * 1024 + p * 512:s * 1024 + (p + 1) * 512],
                        ps[p], bias[:, p:p + 1], None, ADD)
                for p in range(2):  # p0 first: unblocks L1's phase A
                    nc.vector.tensor_mul(
                        h_lf[:, s * 1024 + p * 512:s * 1024 + (p + 1) * 512],
                        to[:, p * 512:(p + 1) * 512],
                        tcell[:, p * 512:(p + 1) * 512])
            st[(0, ch)] = (h_lf, c_lf)

        def emit_level(li, ch):
            prev_h, prev_c = st.pop((li - 1, ch))
            if li == 1:
                h1 = lvl.tile([128, 2048], F16, name="h1", tag="h1", bufs=2)
                c1 = lvl.tile([128, 2048], F16, name="c1", tag="c1", bufs=2)
                cv = prev_c.rearrange("q (b two m) -> q b two m", b=4, two=2)
                for j in range(2):
                    rhs = [prev_h[:, j * 1024:j * 1024 + 512],
                           prev_h[:, j * 1024 + 512:(j + 1) * 1024],
                           prev_h[:, (j + 2) * 1024:(j + 2) * 1024 + 512],
                           prev_h[:, (j + 2) * 1024 + 512:(j + 3) * 1024]]
                    couts = [c1[:, j * 1024:j * 1024 + 512],
                             c1[:, j * 1024 + 512:(j + 1) * 1024]]
                    compose(rhs, cv[:, j], cv[:, j + 2], couts, 512)
                    for p in range(2):
                        nc.scalar.activation(
                            h1[:, j * 1024 + p * 512:j * 1024 + (p + 1) * 512],
                            couts[p], TANH)
                st[(1, ch)] = (h1, c1)
            elif li == 2:
                h2 = lvl.tile([128, 1024], F16, name="h2", tag="h2", bufs=2)
                c2 = lvl.tile([128, 1024], F16, name="c2", tag="c2", bufs=2)
                cv = prev_c.rearrange("q (b two m) -> q b two m", b=2, two=2)
                rhs = [prev_h[:, 0:512], prev_h[:, 512:1024],
                       prev_h[:, 1024:1536], prev_h[:, 1536:2048]]
                couts = [c2[:, 0:512], c2[:, 512:1024]]
                compose(rhs, cv[:, 0], cv[:, 1], couts, 512)
                for p in range(2):
                    nc.scalar.activation(h2[:, p * 512:(p + 1) * 512],
                                         couts[p], TANH)
                st[(2, ch)] = (h2, c2)
            else:  # li == 3: park position-major into the chunk's group
                g = 0 if ch < 4 else (1 if ch < 7 else 2)
                T = GRP_T[g]
                tg = (ch - GRP_CH[g][0]) * T_CHUNK
                cv = prev_c.rearrange("q (two m) -> q two m", two=2)
                rhs = [prev_h[:, 0:256], prev_h[:, 512:768],
                       prev_h[:, 256:512], prev_h[:, 768:1024]]
                pkv = pk_c[g].rearrange("q (two pos t) -> q two pos t",
                                        two=2, pos=32)
                couts = [pkv[:, 0, :, tg:tg + 8], pkv[:, 1, :, tg:tg + 8]]
                compose(rhs, cv[:, :, 0:256], cv[:, :, 256:512], couts, 256)

        def park_tanh(g):
            half = 32 * GRP_T[g]
            for p in range(2):  # p0 first: unblocks L4's phase A
                nc.scalar.activation(pk_h[g][:, p * half:(p + 1) * half],
                                     pk_c[g][:, p * half:(p + 1) * half],
                                     TANH)

        tl = {}  # (level, g) -> (h, c) tail tiles

        def tail_level(li, g):
            if li == 4:
                hp, cp = pk_h[g], pk_c[g]
            else:
                hp, cp = tl.pop((li - 1, g))
            n = GRP_T[g] << (8 - li)     # nodes this level (per group)
            N = 2 * n                    # nodes per half in prev level
            rhs = [hp[:, 0:n], hp[:, N:N + n],
                   hp[:, n:N], hp[:, N + n:2 * N]]
            cv = cp.rearrange("q (two m) -> q two m", two=2)
            lc, rc = cv[:, :, 0:n], cv[:, :, n:N]
            if li < 8:
                nh = lvl.tile([128, 2 * n], F16, name=f"tl{li}h{g}",
                              tag=f"tl{li}h{g}")
                ncr = lvl.tile([128, 2 * n], F16, name=f"tl{li}c{g}",
                               tag=f"tl{li}c{g}")
                couts = [ncr[:, 0:n], ncr[:, n:2 * n]]
                compose(rhs, lc, rc, couts, n)
                for p in range(2):
                    nc.scalar.activation(nh[:, p * n:(p + 1) * n],
                                         couts[p], TANH)
                tl[(li, g)] = (nh, ncr)
            else:                        # roots: n = GRP_T, fp32 staging
                stc = lvl.tile([128, 2 * n], F32, name=f"stc{g}",
                               tag=f"stc{g}")
                sth = lvl.tile([128, 2 * n], F32, name=f"sth{g}",
                               tag=f"sth{g}")
                compose(rhs, lc, rc, [stc[:, 0:n], stc[:, n:2 * n]], n)
                nc.scalar.activation(sth[:], stc[:], TANH)
                base = GRP_BASE[g]
                for p in range(2):
                    nc.sync.dma_start(
                        outc_d[p * 128:(p + 1) * 128, base:base + n],
                        stc[:, p * n:(p + 1) * n])
                    nc.sync.dma_start(
                        outh_d[p * 128:(p + 1) * 128, base:base + n],
                        sth[:, p * n:(p + 1) * n])

        # --- software-pipelined schedule: group-0 tail overlaps chunks 6-7
        for ch in range(N_CHUNKS):
            emit_leaf(ch)
            if ch == 0:
                for k in range(4):
                    nc.scalar.dma_start(wc[k][:],
                                        wc_d[k * 128:(k + 1) * 128, :])
            if ch >= 1:
                emit_level(2, ch - 1)
            emit_level(1, ch)
            if ch >= 2:
                emit_level(3, ch - 2)
            if ch == 6:
                park_tanh(0)
                tail_level(4, 0)
            if ch == 7:
                tail_level(5, 0)
                tail_level(6, 0)
        emit_level(2, N_CHUNKS - 1)   # L2(7)
        emit_level(3, N_CHUNKS - 2)   # L3(6) -> park g1 complete
        tail_level(7, 0)
        emit_level(3, N_CHUNKS - 1)   # L3(7) -> park g2 complete
        park_tanh(1)
        tail_level(4, 1)
        park_tanh(2)
        tail_level(8, 0)
        tail_level(4, 2)
        tail_level(5, 1)
        tail_level(5, 2)
        tail_level(6, 1)
        tail_level(6, 2)
        tail_level(7, 1)
        tail_level(7, 2)
        tail_level(8, 1)
        tail_level(8, 2)

    nc.compile()
    return nc


def _bitrev8(x):
    return int(f"{x:08b}"[::-1], 2)


def _prep_inputs(embs, cx_w, cx_b, ox_w, ox_b, lh_w, lh_b, rh_w, rh_b):
    f16 = np.float16
    w_leaf = np.ascontiguousarray(
        np.concatenate([cx_w, ox_w], axis=0).T).astype(f16)       # [300, 512]
    w_leaf2 = np.zeros((128, 2 * M), f16)
    w_leaf2[0:44] = w_leaf[256:300]
    w_leaf2[64:108] = w_leaf[256:300]
    w_comp = np.ascontiguousarray(np.concatenate(
        [lh_w.reshape(4 * M, M).T, rh_w.reshape(4 * M, M).T],
        axis=0)).astype(f16)                                      # [512, 1024]
    b_all = np.concatenate(
        [cx_b, ox_b, (lh_b + rh_b).reshape(-1)]).astype(np.float32)
    biases = np.ascontiguousarray(b_all.reshape(12, 128).T)       # [128, 12]
    brperm = np.array([_bitrev8(p) for p in range(L)])
    embs_f = np.asarray(embs, np.float32)
    in_maps = []
    for c in range(N_CORES):
        # [chunk, tree, leaf, IN] -> bit-reverse leaves -> position-major
        E = embs_f[c * BC:(c + 1) * BC].reshape(N_CHUNKS, T_CHUNK, L, IN)
        E = E[:, :, brperm, :].transpose(0, 2, 1, 3).reshape(LEAF_ROWS, IN)
        xt = np.ascontiguousarray(E.T.astype(f16))                # [300, 16384]
        in_maps.append({"embs_t": xt,
                        "w_leaf": w_leaf, "w_leaf2": w_leaf2,
                        "w_comp": w_comp, "biases": biases})
    return in_maps


def kernel(embs, cx_w, cx_b, ox_w, ox_b, lh_w, lh_b, rh_w, rh_b):
    global LAST_RESULTS
    from concourse.bass_utils import run_bass_kernel_spmd

    if "nc" not in _CACHE:
        _CACHE["nc"] = _build()
    nc = _CACHE["nc"]
    in_maps = _prep_inputs(embs, cx_w, cx_b, ox_w, ox_b,
                           lh_w, lh_b, rh_w, rh_b)
    res = run_bass_kernel_spmd(nc, in_maps, core_ids=list(range(N_CORES)))
    LAST_RESULTS = res
    c_out = np.empty((B, M), np.float32)
    h_out = np.empty((B, M), np.float32)
    for c in range(N_CORES):
        c_out[c * BC:(c + 1) * BC] = res.results[c]["out_c"].T
        h_out[c * BC:(c + 1) * BC] = res.results[c]["out_h"].T
    return c_out, h_out


# revision 10
# speedup vs baseline: 1.2224x; 1.2224x over previous
"""BinaryTreeLSTM Trainium2 kernel — data-parallel over the batch (tree) axis.

v2: fp16 datapath + bit-reversed position-major layout.

Layout: the host permutes each tree's 256 leaves into bit-reversed order
and interleaves the 8 trees of a chunk (column = position*8 + tree).  With
bit-reversed storage, the children of the node at position p of level l+1
are at positions (p, p + N/2) of level l — so every level's lh/rh operands
are *contiguous halves* instead of stride-2 slices, and the level outputs
are written contiguously.  This keeps every DVE tensor_tensor op on dense
step-1 fp16 data (2x_1P mode) and every matmul moving operand dense.

dtypes: all matmul operands, gates, h and c are fp16 (e5m10) — ~4x lower
end-to-end error than the bf16 variant (sim: 3.0e-3 vs 1.14e-2) because
the c-accumulation chain keeps 10 mantissa bits.  PSUM and biases fp32.

Per core (64 trees): leaf GEMM (K=300 -> 512 feats = [c|o]), then 8
levelized compose GEMMs (K=512 = [lh|rh] feats -> 1024 = 4 gates x 256).
Chunks of 8 trees run leaf+L1..L3 software-pipelined (leaf ch | L2 ch-1 |
L1 ch | L3 ch-2); L3 parks position-major per 32-tree group; the group-0
tail (L4..L8) interleaves with chunks 6-7 so only group 1's tail runs
after the main loop.

Startup: DVE memset -> dummy sigmoid (preloads the sigmoid+tanh ACT table
set during the first DMA wait) -> dummy matmuls (PE HAM warm-up); the
first leaf sub-chunk's embs DMA is split into 4 pieces so the first real
matmul starts as early as possible.
"""

import sys

if "/opt/trn_rl_repo" not in sys.path:
    sys.path.insert(0, "/opt/trn_rl_repo")

from contextlib import ExitStack

import numpy as np

N_CORES = 8
B, L, IN, M = 512, 256, 300, 256
BC = B // N_CORES            # trees per core
LEAF_ROWS = BC * L           # 16384 leaf columns per core
T_CHUNK = 8                  # trees per chunk
N_CHUNKS = BC // T_CHUNK     # 8
CHUNK_LEAF = T_CHUNK * L     # 2048 leaf columns per chunk

_CACHE = {}
LAST_RESULTS = None


def _build():
    import concourse.bass as bass  # noqa: F401
    import concourse.tile as tile
    from concourse import bacc, mybir

    F32 = mybir.dt.float32
    F16 = mybir.dt.float16
    SIG = mybir.ActivationFunctionType.Sigmoid
    TANH = mybir.ActivationFunctionType.Tanh
    ADD = mybir.AluOpType.add

    nc = bacc.Bacc("TRN2", target_bir_lowering=False, debug=False,
                   num_devices=N_CORES)
    emb_d = nc.dram_tensor("embs_t", [IN, LEAF_ROWS], F16,
                           kind="ExternalInput").ap()
    wl_d = nc.dram_tensor("w_leaf", [IN, 2 * M], F16,
                          kind="ExternalInput").ap()
    wc_d = nc.dram_tensor("w_comp", [2 * M, 4 * M], F16,
                          kind="ExternalInput").ap()
    wl2_d = nc.dram_tensor("w_leaf2", [128, 2 * M], F16,
                           kind="ExternalInput").ap()
    b_d = nc.dram_tensor("biases", [128, 12], F32, kind="ExternalInput").ap()
    outc_d = nc.dram_tensor("out_c", [M, BC], F32, kind="ExternalOutput").ap()
    outh_d = nc.dram_tensor("out_h", [M, BC], F32, kind="ExternalOutput").ap()

    with tile.TileContext(nc) as tc, ExitStack() as ctx:
        wpool = ctx.enter_context(tc.tile_pool(name="w", bufs=1))
        xpool = ctx.enter_context(tc.tile_pool(name="x", bufs=5))
        lvl = ctx.enter_context(tc.tile_pool(name="lvl", bufs=1))
        gp = ctx.enter_context(tc.tile_pool(name="g", bufs=2))
        pp = ctx.enter_context(tc.tile_pool(name="ps", bufs=1, space="PSUM"))

        # --- weights / biases (resident); issued on the ACT HWDGE queue so
        # the SP queue leads with the first leaf's embs tiles ---
        wl = [wpool.tile([128, 2 * M], F16, name=f"wl{k}", tag=f"wl{k}")
              for k in range(2)]
        nc.scalar.dma_start(wl[0][:], wl_d[0:128, :])
        nc.scalar.dma_start(wl[1][:], wl_d[128:256, :])
        wl2 = wpool.tile([128, 2 * M], F16, name="wl2", tag="wl2")
        nc.scalar.dma_start(wl2[:], wl2_d[:, :])
        wc = [wpool.tile([128, 4 * M], F16, name=f"wc{k}", tag=f"wc{k}")
              for k in range(4)]
        bias = wpool.tile([128, 12], F32, name="bias", tag="bias")
        nc.scalar.dma_start(bias[:], b_d[:, :])

        # PSUM: eight [128, 512] tiles = one bank each
        def ps_tile(i, n=512):
            t = pp.tile([128, 512], F32, name=f"ps{i}", tag=f"ps{i}")
            return t[:, :n] if n < 512 else t

        # --- warm-up: memset on DVE; dummy sigmoid preloads the
        # sigmoid_and_others ACT table set (covers tanh too, ~2.7us)
        # during the startup DMA wait; dummy matmuls on banks 4-7 engage
        # the PE HAM clock gate ---
        dummy = wpool.tile([128, 512], F16, name="dummy", tag="dummy")
        nc.vector.memset(dummy[:], 0.0)
        actpre = wpool.tile([128, 1], F32, name="actpre", tag="actpre")
        nc.scalar.activation(actpre[:], dummy[:, 0:1], SIG)
        for i in range(7):
            nc.tensor.matmul(ps_tile(4 + i % 4), dummy[:, 0:128],
                             dummy[:], start=True, stop=True)

        # --- parked L3 state: position-major per 32-tree group:
        # col = half*1024 + pos*32 + tree_in_group ---
        pk_h = [lvl.tile([128, 2048], F16, name=f"pkh{g}", tag=f"pkh{g}")
                for g in range(2)]
        pk_c = [lvl.tile([128, 2048], F16, name=f"pkc{g}", tag=f"pkc{g}")
                for g in range(2)]

        # compose m-tile index: mt = gate*2 + part_half, gates (i,lf,rf,u).
        # Drain order (i0,i1,u0,u1,...) lets the DVE c-chain start after
        # four gate activations.
        MT_ORDER = (0, 1, 6, 7, 2, 3, 4, 5)
        GATE_OF_MT = (0, 0, 1, 1, 2, 2, 3, 3)

        def compose(rhs, lc, rc, cout, n):
            """One compose call (n <= 512 output nodes).

            rhs: 4 moving APs [128, n] (lh p0, lh p1, rh p0, rh p1);
            lc/rc: [128, 2, n] APs; cout: [128, 2, n]-shaped AP or list of
            2 per-half APs."""
            ps = [ps_tile(mt, n) for mt in range(8)]
            for mt in MT_ORDER:
                for k in range(4):
                    nc.tensor.matmul(ps[mt],
                                     wc[k][:, mt * 128:(mt + 1) * 128],
                                     rhs[k], start=(k == 0), stop=(k == 3))
            g = [None] * 4
            for mt in MT_ORDER:
                gate, p = mt >> 1, mt & 1
                if g[gate] is None:
                    g[gate] = gp.tile([128, 2, 512], F16, name=f"g{gate}",
                                      tag=f"g{gate}", bufs=3)
                fn = TANH if gate == 3 else SIG
                nc.scalar.activation(g[gate][:, p, :n], ps[mt], fn,
                                     bias=bias[:, 4 + mt:5 + mt])
            t1 = gp.tile([128, 2, 512], F16, name="t1", tag="t1", bufs=2)
            nc.vector.tensor_mul(t1[:, :, :n], g[0][:, :, :n],
                                 g[3][:, :, :n])
            t2 = gp.tile([128, 2, 512], F16, name="t2", tag="t2", bufs=2)
            nc.vector.tensor_mul(t2[:, :, :n], g[1][:, :, :n], lc)
            t3 = gp.tile([128, 2, 512], F16, name="t3", tag="t3", bufs=2)
            nc.vector.tensor_mul(t3[:, :, :n], g[2][:, :, :n], rc)
            s12 = gp.tile([128, 2, 512], F16, name="s12", tag="s12", bufs=2)
            nc.vector.tensor_add(s12[:, :, :n], t1[:, :, :n], t2[:, :, :n])
            if isinstance(cout, list):
                for p in range(2):
                    a, b = s12[:, p, :n], t3[:, p, :n]
                    if cout[p].ndim == 3:  # park layout [128, pos, tree]
                        td = cout[p].shape[2]
                        a = a.rearrange("q (a b) -> q a b", b=td)
                        b = b.rearrange("q (a b) -> q a b", b=td)
                    nc.vector.tensor_add(cout[p], a, b)
            else:
                nc.vector.tensor_add(cout, s12[:, :, :n], t3[:, :, :n])

        st = {}  # (level, ch) -> (h_tile, c_tile)

        def emit_leaf(ch):
            h_lf = lvl.tile([128, 4096], F16, name="hlf", tag="hlf", bufs=2)
            c_lf = lvl.tile([128, 4096], F16, name="clf", tag="clf", bufs=2)
            for s in range(4):
                col0 = ch * CHUNK_LEAF + s * 512
                xk01 = xpool.tile([128, 2, 512], F16, name="xk01",
                                  tag="xk01")
                nc.sync.dma_start(xk01[:, 0, :], emb_d[0:128, col0:col0 + 512])
                nc.sync.dma_start(xk01[:, 1, :],
                                  emb_d[128:256, col0:col0 + 512])
                xk2 = xpool.tile([128, 512], F16, name="xk2", tag="xk2")
                nc.sync.dma_start(xk2[0:44, :],
                                  emb_d[256:300, col0:col0 + 512])
                nc.sync.dma_start(xk2[64:108, :],
                                  emb_d[256:300, col0:col0 + 512])
                pso = 4 * (s % 2)
                ps = [ps_tile(pso + mt) for mt in range(4)]
                # k-outer so the first matmuls only need the first DMA piece
                for k in range(2):
                    for mt in range(4):
                        nc.tensor.matmul(
                            ps[mt], wl[k][:, mt * 128:(mt + 1) * 128],
                            xk01[:, k, :], start=(k == 0), stop=False)
                # K=44 tail: two m-tiles in disjoint PE row groups
                for mtp in (0, 2):
                    nc.tensor.matmul(
                        ps[mtp], wl2[0:44, mtp * 128:(mtp + 1) * 128],
                        xk2[0:44, :], start=False, stop=True,
                        tile_position=(0, 0))
                    nc.tensor.matmul(
                        ps[mtp + 1],
                        wl2[64:108, (mtp + 1) * 128:(mtp + 2) * 128],
                        xk2[64:108, :], start=False, stop=True,
                        tile_position=(64, 0))
                tcell = gp.tile([128, 1024], F16, name="lftc", tag="lftc",
                                bufs=2)
                to = gp.tile([128, 1024], F16, name="lfto", tag="lfto",
                             bufs=2)
                for p in range(2):
                    nc.scalar.activation(tcell[:, p * 512:(p + 1) * 512],
                                         ps[p], TANH, bias=bias[:, p:p + 1])
                    nc.scalar.activation(to[:, p * 512:(p + 1) * 512],
                                         ps[2 + p], SIG,
                                         bias=bias[:, 2 + p:3 + p])
                    nc.vector.tensor_scalar(
                        c_lf[:, s * 1024 + p * 512:s * 1024 + (p + 1) * 512],
                        ps[p], bias[:, p:p + 1], None, ADD)
                nc.vector.tensor_mul(h_lf[:, s * 1024:(s + 1) * 1024],
                                     to[:], tcell[:])
            st[(0, ch)] = (h_lf, c_lf)

        def emit_level(li, ch):
            prev_h, prev_c = st.pop((li - 1, ch))
            if li == 1:
                h1 = lvl.tile([128, 2048], F16, name="h1", tag="h1", bufs=2)
                c1 = lvl.tile([128, 2048], F16, name="c1", tag="c1", bufs=2)
                cv = prev_c.rearrange("q (b two m) -> q b two m", b=4, two=2)
                ov = c1.rearrange("q (b two m) -> q b two m", b=2, two=2)
                for j in range(2):
                    rhs = [prev_h[:, j * 1024:j * 1024 + 512],
                           prev_h[:, j * 1024 + 512:(j + 1) * 1024],
                           prev_h[:, (j + 2) * 1024:(j + 2) * 1024 + 512],
                           prev_h[:, (j + 2) * 1024 + 512:(j + 3) * 1024]]
                    compose(rhs, cv[:, j], cv[:, j + 2], ov[:, j], 512)
                    nc.scalar.activation(h1[:, j * 1024:(j + 1) * 1024],
                                         c1[:, j * 1024:(j + 1) * 1024],
                                         TANH)
                st[(1, ch)] = (h1, c1)
            elif li == 2:
                h2 = lvl.tile([128, 1024], F16, name="h2", tag="h2", bufs=2)
                c2 = lvl.tile([128, 1024], F16, name="c2", tag="c2", bufs=2)
                cv = prev_c.rearrange("q (b two m) -> q b two m", b=2, two=2)
                rhs = [prev_h[:, 0:512], prev_h[:, 512:1024],
                       prev_h[:, 1024:1536], prev_h[:, 1536:2048]]
                compose(rhs, cv[:, 0], cv[:, 1],
                        c2.rearrange("q (two m) -> q two m", two=2), 512)
                nc.scalar.activation(h2[:], c2[:], TANH)
                st[(2, ch)] = (h2, c2)
            else:  # li == 3: park position-major per 32-tree group
                g, tg = ch // 4, (ch % 4) * 8
                cv = prev_c.rearrange("q (two m) -> q two m", two=2)
                rhs = [prev_h[:, 0:256], prev_h[:, 512:768],
                       prev_h[:, 256:512], prev_h[:, 768:1024]]
                pkv = pk_c[g].rearrange("q (two pos t) -> q two pos t",
                                        two=2, pos=32)
                couts = [pkv[:, 0, :, tg:tg + 8], pkv[:, 1, :, tg:tg + 8]]
                compose(rhs, cv[:, :, 0:256], cv[:, :, 256:512], couts, 256)

        def park_tanh(g):
            nc.scalar.activation(pk_h[g][:], pk_c[g][:], TANH)

        tl = {}  # (level, g) -> (h, c) tail tiles

        def tail_level(li, g):
            if li == 4:
                hp, cp = pk_h[g], pk_c[g]
            else:
                hp, cp = tl.pop((li - 1, g))
            n = 512 >> (li - 4)          # nodes this level (per group)
            N = 2 * n                    # nodes per half in prev level
            rhs = [hp[:, 0:n], hp[:, N:N + n],
                   hp[:, n:N], hp[:, N + n:2 * N]]
            cv = cp.rearrange("q (two m) -> q two m", two=2)
            lc, rc = cv[:, :, 0:n], cv[:, :, n:N]
            if li < 8:
                nh = lvl.tile([128, 2 * n], F16, name=f"tl{li}h{g}",
                              tag=f"tl{li}h{g}")
                ncr = lvl.tile([128, 2 * n], F16, name=f"tl{li}c{g}",
                               tag=f"tl{li}c{g}")
                compose(rhs, lc, rc,
                        ncr.rearrange("q (two m) -> q two m", two=2), n)
                nc.scalar.activation(nh[:], ncr[:], TANH)
                tl[(li, g)] = (nh, ncr)
            else:                        # roots: n = 32, fp32 staging
                stc = lvl.tile([128, 64], F32, name=f"stc{g}", tag=f"stc{g}")
                sth = lvl.tile([128, 64], F32, name=f"sth{g}", tag=f"sth{g}")
                compose(rhs, lc, rc,
                        stc.rearrange("q (two m) -> q two m", two=2), n)
                nc.scalar.activation(sth[:], stc[:], TANH)
                for p in range(2):
                    nc.sync.dma_start(
                        outc_d[p * 128:(p + 1) * 128, g * 32:(g + 1) * 32],
                        stc[:, p * 32:(p + 1) * 32])
                    nc.sync.dma_start(
                        outh_d[p * 128:(p + 1) * 128, g * 32:(g + 1) * 32],
                        sth[:, p * 32:(p + 1) * 32])

        # --- software-pipelined schedule: group-0 tail overlaps chunks 6-7
        for ch in range(N_CHUNKS):
            emit_leaf(ch)
            if ch == 0:
                for k in range(4):
                    nc.scalar.dma_start(wc[k][:],
                                        wc_d[k * 128:(k + 1) * 128, :])
            if ch >= 1:
                emit_level(2, ch - 1)
            emit_level(1, ch)
            if ch >= 2:
                emit_level(3, ch - 2)
            if ch == 6:
                park_tanh(0)
                tail_level(4, 0)
            if ch == 7:
                tail_level(5, 0)
                tail_level(6, 0)
        emit_level(2, N_CHUNKS - 1)
        emit_level(3, N_CHUNKS - 2)
        tail_level(7, 0)
        emit_level(3, N_CHUNKS - 1)
        park_tanh(1)
        tail_level(8, 0)
        tail_level(4, 1)
        tail_level(5, 1)
        tail_level(6, 1)
        tail_level(7, 1)
        tail_level(8, 1)

    nc.compile()
    return nc


def _bitrev8(x):
    return int(f"{x:08b}"[::-1], 2)


def _prep_inputs(embs, cx_w, cx_b, ox_w, ox_b, lh_w, lh_b, rh_w, rh_b):
    f16 = np.float16
    w_leaf = np.ascontiguousarray(
        np.concatenate([cx_w, ox_w], axis=0).T).astype(f16)       # [300, 512]
    w_leaf2 = np.zeros((128, 2 * M), f16)
    w_leaf2[0:44] = w_leaf[256:300]
    w_leaf2[64:108] = w_leaf[256:300]
    w_comp = np.ascontiguousarray(np.concatenate(
        [lh_w.reshape(4 * M, M).T, rh_w.reshape(4 * M, M).T],
        axis=0)).astype(f16)                                      # [512, 1024]
    b_all = np.concatenate(
        [cx_b, ox_b, (lh_b + rh_b).reshape(-1)]).astype(np.float32)
    biases = np.ascontiguousarray(b_all.reshape(12, 128).T)       # [128, 12]
    brperm = np.array([_bitrev8(p) for p in range(L)])
    embs_f = np.asarray(embs, np.float32)
    in_maps = []
    for c in range(N_CORES):
        # [chunk, tree, leaf, IN] -> bit-reverse leaves -> position-major
        E = embs_f[c * BC:(c + 1) * BC].reshape(N_CHUNKS, T_CHUNK, L, IN)
        E = E[:, :, brperm, :].transpose(0, 2, 1, 3).reshape(LEAF_ROWS, IN)
        xt = np.ascontiguousarray(E.T.astype(f16))                # [300, 16384]
        in_maps.append({"embs_t": xt,
                        "w_leaf": w_leaf, "w_leaf2": w_leaf2,
                        "w_comp": w_comp, "biases": biases})
    return in_maps


def kernel(embs, cx_w, cx_b, ox_w, ox_b, lh_w, lh_b, rh_w, rh_b):
    global LAST_RESULTS
    from concourse.bass_utils import run_bass_kernel_spmd

    if "nc" not in _CACHE:
        _CACHE["nc"] = _build()
    nc = _CACHE["nc"]
    in_maps = _prep_inputs(embs, cx_w, cx_b, ox_w, ox_b,
                           lh_w, lh_b, rh_w, rh_b)
    res = run_bass_kernel_spmd(nc, in_maps, core_ids=list(range(N_CORES)))
    LAST_RESULTS = res
    c_out = np.empty((B, M), np.float32)
    h_out = np.empty((B, M), np.float32)
    for c in range(N_CORES):
        c_out[c * BC:(c + 1) * BC] = res.results[c]["out_c"].T
        h_out[c * BC:(c + 1) * BC] = res.results[c]["out_h"].T
    return c_out, h_out
